# revision 69
# baseline (speedup 1.0000x reference)
"""BERT-encoder (12-layer) forward as a Bass/Tile kernel on 8 TRN2 NeuronCores.

Sharding: pure data-parallel over batch — B=16 sequences, 2 per core.
On-device layout: activations are kept feature-major (x^T : [E, tokens]) so
they can feed the TensorEngine directly (contraction dim on partitions).
All matmul operands are bf16 (enables TensorE fast-weight-load and keeps the
clock-gate warm); the residual stream hT stays fp32 (with a bf16 shadow hTb
feeding the LN-statistics matmuls), and all PSUM accumulation is fp32.
Per-layer weights stream from HBM as bf16 (~14 MB/layer) overlapped with
compute; the 12-layer loop is a hardware loop with staggered semaphore reset
so engines pipeline across the layer boundary.

Host side only reshapes/transposes/pads the input tensors into DMA-friendly
layouts (pure marshalling) — all arithmetic happens on-device.
"""

import numpy as np
import ml_dtypes

import concourse.bass as bass
import concourse.mybir as mybir
import concourse.tile as tile
from concourse import bacc
from concourse.bass import ds

f32 = mybir.dt.float32
f32r = mybir.dt.float32r
bf16 = mybir.dt.bfloat16
i32 = mybir.dt.int32
AF = mybir.ActivationFunctionType
ALU = mybir.AluOpType

# model dims
L, H, E, D, F, V, T, B = 12, 12, 768, 64, 3072, 30522, 513, 16
NCORE = 8
BP = B // NCORE            # sequences per core = 2
NT = BP * T                # tokens per core = 1026
EC = E // 128              # 6 chunks of the embedding dim
FC = F // 128              # 24 chunks of the mlp dim
HP = H // 2                # 6 head-pairs
TCH = [(0, 342), (342, 342), (684, 342)]   # token chunks (all >=256 for f32r)
SQRT_E = float(np.sqrt(E))
EPS = 1e-5


def _stiles(b):
    """Per-sequence 128-row s-tiles: (col in [0,NT), rows)."""
    return [(b * T + k * 128, min(128, T - k * 128)) for k in range(5)]


def _tok_tiles():
    """All (b, st, col, rows) token tiles."""
    out = []
    for b in range(BP):
        for k, (col, rows) in enumerate(_stiles(b)):
            out.append((b, k, col, rows))
    return out


def build(nl=L, hw_loop=True, stage=4):
    """Build the Bass module. Returns nc.

    stage (debug, use with nl=1 unrolled): 1=stop after LN1, 2=after
    attention (pre-Wo), 3=after Wo+residual, 4=full layer."""
    nc = bacc.Bacc("TRN2", target_bir_lowering=False, debug=False,
                   num_devices=NCORE)

    # ---------------- DRAM I/O (host-marshalled layouts) ----------------
    idx_d = nc.dram_tensor("idx", (2 * 5, 128), i32, kind="ExternalInput")
    tok_d = nc.dram_tensor("tok_emb", (V, E), f32, kind="ExternalInput")
    posT_d = nc.dram_tensor("posT", (EC, 128, T), f32, kind="ExternalInput")
    seg_d = nc.dram_tensor("seg", (128, EC), f32, kind="ExternalInput")

    wq_d = nc.dram_tensor("wq", (L, HP, 128, E), bf16, kind="ExternalInput")
    wk_d = nc.dram_tensor("wk", (L, HP, 128, E), bf16, kind="ExternalInput")
    wv_d = nc.dram_tensor("wv", (L, 128, EC * E), bf16, kind="ExternalInput")
    wo_d = nc.dram_tensor("wo", (L, EC, 128, E), bf16, kind="ExternalInput")
    w1_d = nc.dram_tensor("w1", (L, FC, 128, E), bf16, kind="ExternalInput")
    w2_d = nc.dram_tensor("w2", (L, EC, 128, FC * 128), bf16, kind="ExternalInput")

    bq_d = nc.dram_tensor("bq", (L, 128, HP), f32, kind="ExternalInput")
    bk_d = nc.dram_tensor("bk", (L, 128, HP), f32, kind="ExternalInput")
    bv_d = nc.dram_tensor("bv", (L, 1, E), bf16, kind="ExternalInput")
    bo_d = nc.dram_tensor("bo", (L, 128, EC), f32, kind="ExternalInput")
    b1_d = nc.dram_tensor("b1", (L, 128, FC), f32, kind="ExternalInput")
    b2_d = nc.dram_tensor("b2", (L, 128, EC), f32, kind="ExternalInput")
    g1_d = nc.dram_tensor("g1", (L, 128, EC), f32, kind="ExternalInput")
    be1_d = nc.dram_tensor("be1", (L, 128, EC), f32, kind="ExternalInput")
    g2_d = nc.dram_tensor("g2", (L, 128, EC), f32, kind="ExternalInput")
    be2_d = nc.dram_tensor("be2", (L, 128, EC), f32, kind="ExternalInput")

    ident_d = nc.dram_tensor("ident", (128, 128), f32, kind="ExternalInput")
    onesr_d = nc.dram_tensor("onesr", (128, 128), bf16, kind="ExternalInput")
    onesh_d = nc.dram_tensor("onesh", (128, H), bf16, kind="ExternalInput")
    epsb_d = nc.dram_tensor("epsb", (128, 1), f32, kind="ExternalInput")

    out_d = nc.dram_tensor("out", (NT, E), f32, kind="ExternalOutput")

    def lsl(ap, l_iv, *rest):
        """Slice DRAM ap at layer l (static int or runtime value)."""
        if hw_loop:
            r = ap[(ds(l_iv, 1),) + rest]
            # drop the leading size-1 layer dim
            letters = [chr(ord('b') + i) for i in range(len(r.shape) - 1)]
            spec = "a " + " ".join(letters) + " -> " + "(a " + letters[0] + ") " + " ".join(letters[1:])
            return r.rearrange(spec)
        else:
            return ap[(l_iv,) + rest]

    with tile.TileContext(nc) as tc:
        with tc.tile_pool(name="res_sb", bufs=1) as res:
            # persistent tiles
            hT = [res.tile([128, NT], f32r, name=f"hT{e}") for e in range(EC)]
            # bf16 shadow of hT, kept in sync at every residual update; feeds
            # the LN-statistics matmuls (PE rejects mixed f32r x bf16 inputs).
            hTb = [res.tile([128, NT], bf16, name=f"hTb{e}") for e in range(EC)]
            ident = res.tile([128, 128], f32)
            onesr = res.tile([128, 128], bf16)
            onesh = res.tile([128, H], bf16)
            epsb = res.tile([128, 1], f32)

            nc.sync.dma_start(out=ident[:], in_=ident_d.ap())
            nc.sync.dma_start(out=onesr[:], in_=onesr_d.ap())
            nc.sync.dma_start(out=onesh[:], in_=onesh_d.ap())
            nc.sync.dma_start(out=epsb[:], in_=epsb_d.ap())

            # ---------------- embedding ----------------
            with tc.tile_pool(name="emb_sb", bufs=1) as emb, \
                 tc.tile_pool(name="emb_ps", bufs=4, space="PSUM") as embps:
                posT = [emb.tile([128, T], f32, name=f"posT{e}") for e in range(EC)]
                seg_sb = emb.tile([128, EC], f32)
                idx_sb = emb.tile([128, 2 * 5], i32)
                nc.sync.dma_start(out=seg_sb[:], in_=seg_d.ap())
                nc.sync.dma_start(out=idx_sb[:], in_=idx_d.ap().rearrange("t p -> p t"))
                for e in range(EC):
                    nc.sync.dma_start(out=posT[e][:], in_=posT_d.ap()[e])
                    # add segment embedding (per-partition bias), in place
                    nc.scalar.activation(out=posT[e][:], in_=posT[e][:],
                                         func=AF.Identity,
                                         bias=seg_sb[:, e:e + 1])
                for (b, st, col, rows) in _tok_tiles():
                    tt = b * 5 + st
                    g = emb.tile([128, E], f32, tag="gath", bufs=3)
                    nc.gpsimd.indirect_dma_start(
                        out=g[:], out_offset=None,
                        in_=tok_d.ap(),
                        in_offset=bass.IndirectOffsetOnAxis(
                            ap=idx_sb[:, tt:tt + 1], axis=0),
                    )
                    for e in range(EC):
                        tp = embps.tile([128, 128], f32, tag="tp")
                        nc.tensor.transpose(out=tp[:], in_=g[:, e * 128:(e + 1) * 128],
                                            identity=ident[:])
                        nc.vector.tensor_tensor(
                            out=hT[e][:, col:col + rows],
                            in0=tp[:, :rows],
                            in1=posT[e][:, st * 128:st * 128 + rows],
                            op=ALU.add)
                        nc.scalar.activation(
                            out=hTb[e][:, col:col + rows],
                            in_=hT[e][:, col:col + rows].bitcast(f32),
                            func=AF.Copy)

            # ---------------- layers ----------------
            def layer_body(l_iv):
                with tc.tile_pool(name="ln_sb", bufs=1) as lnp:
                    nT = [lnp.tile([128, NT], bf16, name=f"nT{e}") for e in range(EC)]
                    # per-layer params
                    par = lnp.tile([128, 8 * EC + FC], f32, name="par")
                    # columns: [bq 6][bk 6][bo 6][b2 6][g1 6][be1 6][g2 6][be2 6][b1 24]
                    nc.sync.dma_start(out=par[:, 0:HP], in_=lsl(bq_d.ap(), l_iv))
                    nc.sync.dma_start(out=par[:, HP:2 * HP], in_=lsl(bk_d.ap(), l_iv))
                    nc.sync.dma_start(out=par[:, 12:18], in_=lsl(bo_d.ap(), l_iv))
                    nc.sync.dma_start(out=par[:, 18:24], in_=lsl(b2_d.ap(), l_iv))
                    nc.sync.dma_start(out=par[:, 24:30], in_=lsl(g1_d.ap(), l_iv))
                    nc.sync.dma_start(out=par[:, 30:36], in_=lsl(be1_d.ap(), l_iv))
                    nc.sync.dma_start(out=par[:, 36:42], in_=lsl(g2_d.ap(), l_iv))
                    nc.sync.dma_start(out=par[:, 42:48], in_=lsl(be2_d.ap(), l_iv))
                    nc.sync.dma_start(out=par[:, 48:48 + FC], in_=lsl(b1_d.ap(), l_iv))
                    bq_c, bk_c = 0, HP
                    bo_c, b2_c = 12, 18
                    g1_c, be1_c, g2_c, be2_c, b1_c = 24, 30, 36, 42, 48
                    bv_sb = lnp.tile([1, E], bf16, name="bv_sb")
                    nc.sync.dma_start(out=bv_sb[:], in_=lsl(bv_d.ap(), l_iv))

                    def layernorm(g_col, b_col, dst):
                        """dst[e] (an AP per e-chunk) = LN(hT)[e] * g + b."""
                        with tc.tile_pool(name="st_ps", bufs=1, space="PSUM") as stps:
                            sums = [stps.tile([128, w], f32, tag=f"sum{i}", name=f"sum{i}")
                                    for i, (c0, w) in enumerate(TCH)]
                            sqs = [stps.tile([128, w], f32, tag=f"sq{i}", name=f"sq{i}")
                                   for i, (c0, w) in enumerate(TCH)]
                            sqt = lnp.tile([128, NT], bf16, tag="sqt", bufs=1)
                            for e in range(EC):
                                nc.scalar.square(out=sqt[:], in_=hTb[e][:])
                                for i, (c0, w) in enumerate(TCH):
                                    nc.tensor.matmul(out=sums[i][:], lhsT=onesr[:],
                                                     rhs=hTb[e][:, c0:c0 + w],
                                                     start=(e == 0), stop=(e == EC - 1))
                                    nc.tensor.matmul(out=sqs[i][:], lhsT=onesr[:],
                                                     rhs=sqt[:, c0:c0 + w],
                                                     start=(e == 0), stop=(e == EC - 1))
                            mean = lnp.tile([128, NT], f32, tag="mean", bufs=1)
                            t1 = lnp.tile([128, NT], f32, tag="t1", bufs=1)
                            rstd = lnp.tile([128, NT], f32, tag="rstd", bufs=1)
                            for i, (c0, w) in enumerate(TCH):
                                sl = slice(c0, c0 + w)
                                nc.scalar.activation(out=mean[:, sl], in_=sums[i][:],
                                                     func=AF.Copy, scale=1.0 / E)
                                # t1 = sum * mean = E * mean^2
                                nc.vector.tensor_tensor(out=t1[:, sl], in0=sums[i][:],
                                                        in1=mean[:, sl], op=ALU.mult)
                                # t1 = sumsq - E*mean^2 = E * var
                                nc.vector.tensor_tensor(out=t1[:, sl], in0=sqs[i][:],
                                                        in1=t1[:, sl], op=ALU.subtract)
                            # rstd_raw = 1/sqrt(E*var + E*eps); true rstd = sqrt(E)*rstd_raw
                            nc.scalar.activation(out=t1[:], in_=t1[:], func=AF.Sqrt,
                                                 bias=epsb[:, 0:1])
                            nc.vector.reciprocal_approx_fast(out=rstd[:], in_=t1[:])
                            for e in range(EC):
                                xm = lnp.tile([128, NT], f32, tag="xm", bufs=2)
                                nc.vector.tensor_tensor(out=xm[:],
                                                        in0=hT[e][:].bitcast(f32),
                                                        in1=mean[:], op=ALU.subtract)
                                nc.vector.scalar_tensor_tensor(
                                    out=xm[:], in0=xm[:], scalar=SQRT_E,
                                    in1=rstd[:], op0=ALU.mult, op1=ALU.mult)
                                nc.scalar.activation(out=dst[e], in_=xm[:],
                                                     func=AF.Identity,
                                                     scale=par[:, g_col + e:g_col + e + 1],
                                                     bias=par[:, b_col + e:b_col + e + 1])

                    # ===== LN1 =====
                    layernorm(g1_c, be1_c, [t[:] for t in nT])
                    if stage == 1:
                        for e in range(EC):
                            nc.vector.tensor_copy(out=hT[e][:], in_=nT[e][:])
                        return

                    # ===== attention =====
                    with tc.tile_pool(name="at_sb", bufs=1) as atp:
                        qT = [atp.tile([128, NT], bf16, name=f"qT{i}") for i in range(HP)]
                        kT = [atp.tile([128, NT], bf16, name=f"kT{i}") for i in range(HP)]
                        vp = [atp.tile([128, H, 65], bf16, name=f"vp{i}")
                              for i in range(len(_tok_tiles()))]
                        oT = [atp.tile([128, NT], bf16, name=f"oT{e}") for e in range(EC)]

                        # --- v projection (token-major, all heads, +bias) ---
                        # vch=0 (heads 0-5, needed from head-pair 0) runs here;
                        # vch=1 (heads 6-11, first needed at head-pair 3) is
                        # deferred into the attention stream as dense filler.
                        wvt = atp.tile([128, EC * E], bf16, name="wvt")
                        nc.sync.dma_start(out=wvt[:], in_=lsl(wv_d.ap(), l_iv))

                        def v_group(pool, tag, bufs, tt_entry, vch):
                            b, st, col, rows = tt_entry
                            tt = b * 5 + st
                            ps = pool.tile([128, 384], f32, tag=tag, bufs=bufs)
                            for e in range(EC):
                                nc.tensor.matmul(
                                    out=ps[:rows, :],
                                    lhsT=nT[e][:, col:col + rows],
                                    rhs=wvt[:, e * E + vch * 384: e * E + (vch + 1) * 384],
                                    start=(e == 0), stop=False)
                            nc.tensor.matmul(
                                out=ps[:rows, :], lhsT=onesr[0:1, 0:rows],
                                rhs=bv_sb[0:1, vch * 384:(vch + 1) * 384],
                                start=False, stop=True)
                            # scatter 6 heads into the 65-col-stride layout
                            nc.vector.tensor_copy(
                                out=vp[tt][:rows, vch * 6:(vch + 1) * 6, 0:64],
                                in_=ps[:rows, :].rearrange("p (h d) -> p h d", d=64))

                        with tc.tile_pool(name="v_ps", bufs=1, space="PSUM") as vpps:
                            for entry in _tok_tiles():
                                tt = entry[0] * 5 + entry[1]
                                # ones columns of v' (65-col-stride layout)
                                nc.sync.dma_start(
                                    out=vp[tt][:, :, 64:65],
                                    in_=onesh_d.ap()[:, :, None])
                                v_group(vpps, "vps", 4, entry, 0)

                        # --- q/k projections interleaved with scores/AV ---
                        # attention matmuls only use half the PE array (64-deep
                        # contraction / 65-wide output), which the HAM activity
                        # monitor reads as idle -> the clock-gate throttles the
                        # whole attention phase to 1.2GHz. Interleaving each
                        # head-pair's full-array projection matmuls into the
                        # stream keeps the activity up. Scores for head h+1 are
                        # emitted before AV of head h so the in-order PE never
                        # stalls on ACT's exp.
                        with tc.tile_pool(name="sc_ps", bufs=1, space="PSUM") as scps:
                            def emit_av(b, h, at_l):
                                hp, rb = h // 2, (h % 2) * 64
                                ops = scps.tile([128, T], f32, tag="ops", bufs=2)
                                for (t0, tw) in ((0, 512), (512, 1)):
                                    for k, (scol, rows) in enumerate(_stiles(b)):
                                        tt = b * 5 + k
                                        nc.tensor.matmul(
                                            out=ops[:65, t0:t0 + tw],
                                            lhsT=vp[tt][0:rows, h, :],
                                            rhs=at_l[k][0][0:rows, t0:t0 + tw],
                                            start=(k == 0), stop=(k == 4))
                                # NB: reciprocal_approx_fast misreads PSUM APs at
                                # partition base 64 on HW — stage via SBUF first.
                                dsb = atp.tile([1, T], f32, tag="dsb", bufs=2)
                                nc.vector.tensor_copy(out=dsb[:], in_=ops[64:65, :])
                                rec = atp.tile([1, T], f32, tag="rec", bufs=2)
                                nc.vector.reciprocal_approx_fast(out=rec[:], in_=dsb[:])
                                recb = atp.tile([64, T], f32, tag="recb", bufs=2)
                                nc.gpsimd.partition_broadcast(recb[:], rec[:])
                                nc.vector.tensor_tensor(
                                    out=oT[hp][rb:rb + 64, b * T:(b + 1) * T],
                                    in0=ops[0:64, :], in1=recb[:], op=ALU.mult)

                            wts = {}

                            def load_wt(hp, which):
                                wt = atp.tile([128, E], bf16, tag="wqk", bufs=4)
                                nc.sync.dma_start(
                                    out=wt[:],
                                    in_=lsl((wq_d, wk_d)[which].ap(), l_iv, hp))
                                wts[(hp, which)] = wt

                            def qk_group(hp, which, i):
                                b_col, dstl = ((bq_c, qT), (bk_c, kT))[which]
                                c0, w = TCH[i]
                                wt = wts[(hp, which)]
                                ps = scps.tile([128, w], f32, tag="sc", bufs=2)
                                for e in range(EC):
                                    nc.tensor.matmul(
                                        out=ps[:], lhsT=wt[:, e * 128:(e + 1) * 128],
                                        rhs=nT[e][:, c0:c0 + w],
                                        start=(e == 0), stop=(e == EC - 1))
                                nc.vector.tensor_scalar_add(
                                    out=dstl[hp][:, c0:c0 + w], in0=ps[:],
                                    scalar1=par[:, b_col + hp:b_col + hp + 1])

                            # prologue: head-pair 0 projections up front
                            for which in (0, 1):
                                load_wt(0, which)
                                for i in range(3):
                                    qk_group(0, which, i)

                            pend = []
                            vrest = list(_tok_tiles())
                            for hp in range(HP):
                                # dense filler queue for this head-pair's span:
                                # next head-pair's projections + deferred vch=1
                                # v groups, spread between attention units so
                                # every HAM window sees full-array activity
                                fillers = []
                                if hp + 1 < HP:
                                    for which in (0, 1):
                                        load_wt(hp + 1, which)
                                        for i in range(3):
                                            fillers.append(
                                                lambda hp=hp, which=which, i=i:
                                                qk_group(hp + 1, which, i))
                                if hp in (1, 2):
                                    for entry in (vrest[:5] if hp == 1 else vrest[5:]):
                                        fillers.append(
                                            lambda entry=entry:
                                            v_group(scps, "sc", 2, entry, 1))
                                nf = len(fillers)
                                units = ((0, 2 * hp), (0, 2 * hp + 1),
                                         (1, 2 * hp), (1, 2 * hp + 1))
                                for ui, (b, h) in enumerate(units):
                                    for fl in fillers[(ui * nf) // 4:((ui + 1) * nf) // 4]:
                                        fl()
                                    rb = (h % 2) * 64
                                    at_l = []
                                    for k, (scol, rows) in enumerate(_stiles(b)):
                                        sc = scps.tile([128, T], f32, tag="sc", bufs=2)
                                        for (t0, tw) in ((0, 512), (512, 1)):
                                            nc.tensor.matmul(
                                                out=sc[:rows, t0:t0 + tw],
                                                lhsT=qT[hp][rb:rb + 64, scol:scol + rows],
                                                rhs=kT[hp][rb:rb + 64, b * T + t0:b * T + t0 + tw],
                                                start=True, stop=True)
                                        a_t = atp.tile([128, T], bf16, tag="at", bufs=15)
                                        nc.scalar.activation(out=a_t[:rows, :], in_=sc[:rows, :],
                                                             func=AF.Exp)
                                        at_l.append((a_t, rows))
                                    pend.append((b, h, at_l))
                                    if len(pend) > 2:
                                        emit_av(*pend.pop(0))
                            while pend:
                                emit_av(*pend.pop(0))

                        if stage == 2:
                            for e in range(EC):
                                nc.vector.tensor_copy(out=hT[e][:], in_=oT[e][:])
                            return

                        # --- Wo + residual ---
                        with tc.tile_pool(name="wo_ps", bufs=1, space="PSUM") as wops:
                            for eo in range(EC):
                                wt = atp.tile([128, E], bf16, tag="wot", bufs=2)
                                nc.sync.dma_start(out=wt[:], in_=lsl(wo_d.ap(), l_iv, eo))
                                for i, (c0, w) in enumerate(TCH):
                                    ps = wops.tile([128, w], f32, tag="wo", bufs=8)
                                    for e in range(EC):
                                        nc.tensor.matmul(
                                            out=ps[:], lhsT=wt[:, e * 128:(e + 1) * 128],
                                            rhs=oT[e][:, c0:c0 + w],
                                            start=(e == 0), stop=(e == EC - 1))
                                    nc.vector.scalar_tensor_tensor(
                                        out=hT[eo][:, c0:c0 + w], in0=ps[:],
                                        scalar=par[:, bo_c + eo:bo_c + eo + 1],
                                        in1=hT[eo][:, c0:c0 + w].bitcast(f32),
                                        op0=ALU.add, op1=ALU.add)
                                    nc.vector.tensor_copy(
                                        out=hTb[eo][:, c0:c0 + w],
                                        in_=hT[eo][:, c0:c0 + w].bitcast(f32))

                    if stage == 3:
                        return

                    # ===== LN2 =====
                    layernorm(g2_c, be2_c, [t[:] for t in nT])

                    # ===== MLP (bf16) =====
                    with tc.tile_pool(name="ml_sb", bufs=1) as mlp:
                        mT = [mlp.tile([128, NT], bf16, name=f"mT{i}") for i in range(FC)]
                        with tc.tile_pool(name="ml_ps", bufs=1, space="PSUM") as mlps:
                            for fm in range(FC):
                                wt = mlp.tile([128, E], bf16, tag="w1t", bufs=3)
                                nc.sync.dma_start(out=wt[:], in_=lsl(w1_d.ap(), l_iv, fm))
                                for i, (c0, w) in enumerate(TCH):
                                    ps = mlps.tile([128, w], f32, tag="m", bufs=4)
                                    for e in range(EC):
                                        nc.tensor.matmul(
                                            out=ps[:], lhsT=wt[:, e * 128:(e + 1) * 128],
                                            rhs=nT[e][:, c0:c0 + w],
                                            start=(e == 0), stop=(e == EC - 1))
                                    # relu(ps + b1) on DVE (ACT is the scarcer engine)
                                    nc.vector.tensor_scalar(
                                        out=mT[fm][:, c0:c0 + w], in0=ps[:],
                                        scalar1=par[:, b1_c + fm:b1_c + fm + 1],
                                        scalar2=0.0, op0=ALU.add, op1=ALU.max)
                            for eo in range(EC):
                                w2t = mlp.tile([128, FC * 128], bf16, tag="w2t", bufs=2)
                                nc.sync.dma_start(out=w2t[:], in_=lsl(w2_d.ap(), l_iv, eo))
                                for i, (c0, w) in enumerate(TCH):
                                    ps = mlps.tile([128, w], f32, tag="o2", bufs=4)
                                    for fc in range(FC):
                                        nc.tensor.matmul(
                                            out=ps[:], lhsT=w2t[:, fc * 128:(fc + 1) * 128],
                                            rhs=mT[fc][:, c0:c0 + w],
                                            start=(fc == 0), stop=(fc == FC - 1))
                                    nc.vector.scalar_tensor_tensor(
                                        out=hT[eo][:, c0:c0 + w], in0=ps[:],
                                        scalar=par[:, b2_c + eo:b2_c + eo + 1],
                                        in1=hT[eo][:, c0:c0 + w].bitcast(f32),
                                        op0=ALU.add, op1=ALU.add)
                                    nc.vector.tensor_copy(
                                        out=hTb[eo][:, c0:c0 + w],
                                        in_=hT[eo][:, c0:c0 + w].bitcast(f32))

            if nl == 0:
                pass
            elif hw_loop:
                ET = mybir.EngineType
                with tc.For_i(0, nl, 1, staggered_reset=True, hint_engines=(
                        ET.PE, ET.Activation, ET.DVE, ET.Pool, ET.SP)) as l_iv:
                    layer_body(l_iv)
            else:
                for l in range(nl):
                    layer_body(l)

            # ---------------- output (transpose back to token-major) ----------------
            with tc.tile_pool(name="fin_sb", bufs=1) as fin, \
                 tc.tile_pool(name="fin_ps", bufs=4, space="PSUM") as finps:
                for (b, st, col, rows) in _tok_tiles():
                    og = fin.tile([128, E], f32, tag="og", bufs=3)
                    for e in range(EC):
                        tp = finps.tile([128, 128], f32, tag="ftp")
                        nc.tensor.transpose(out=tp[:rows, :],
                                            in_=hT[e][:, col:col + rows].bitcast(f32),
                                            identity=ident[:])
                        nc.vector.tensor_copy(out=og[:rows, e * 128:(e + 1) * 128],
                                              in_=tp[:rows, :])
                    nc.sync.dma_start(out=out_d.ap()[col:col + rows, :], in_=og[:rows, :])

    nc.compile()
    return nc


# ---------------------------------------------------------------------------
# host-side marshalling
# ---------------------------------------------------------------------------

def _marshal_shared(inputs, nl):
    """Weights/layouts shared by all cores."""
    f = lambda k: np.asarray(inputs[k], dtype=np.float32)
    Wq, Wk, Wv = f("Wq"), f("Wk"), f("Wv")
    sh = {}
    sh["tok_emb"] = f("tok_emb")
    pos = f("pos_emb")[:T]                                  # [513, E]
    sh["posT"] = np.ascontiguousarray(
        pos.T.reshape(EC, 128, T))                          # [6,128,513]
    sh["seg"] = np.ascontiguousarray(f("seg_emb")[0].reshape(EC, 128).T)

    bf = ml_dtypes.bfloat16

    def qk_arr(w):
        # [L,H,E,D] -> [L, hp, er, ec, (jh d)] -> [L,6,128,768]
        a = w.reshape(L, HP, 2, EC, 128, D).transpose(0, 1, 4, 3, 2, 5)
        return np.ascontiguousarray(a.reshape(L, HP, 128, E)).astype(bf)
    sh["wq"], sh["wk"] = qk_arr(Wq), qk_arr(Wk)
    # Wv: [L,H,E,D] -> [L, er, ec, h, d] -> [L,128, 6*768]
    a = Wv.reshape(L, H, EC, 128, D).transpose(0, 3, 2, 1, 4)
    sh["wv"] = np.ascontiguousarray(a.reshape(L, 128, EC * E)).astype(bf)
    # Wo: [L,E,E] -> [L, eo, er, ec, j]
    a = f("Wo").reshape(L, EC, 128, EC, 128).transpose(0, 3, 2, 1, 4)
    sh["wo"] = np.ascontiguousarray(a.reshape(L, EC, 128, E)).astype(bf)
    # W1: [L,E,F] -> [l, fm, p, er, m] -> (L, FC, 128, E)
    a = f("W1").reshape(L, EC, 128, FC, 128).transpose(0, 3, 2, 1, 4)
    sh["w1"] = np.ascontiguousarray(a.reshape(L, FC, 128, E)).astype(bf)
    # W2: [L,F,E] -> [l, eo, p, fc, m] -> (L, EC, 128, FC*128)  (one DMA per eo)
    a = f("W2").reshape(L, FC, 128, EC, 128).transpose(0, 3, 2, 1, 4)
    sh["w2"] = np.ascontiguousarray(a.reshape(L, EC, 128, FC * 128)).astype(bf)
    # biases
    sh["bq"] = np.ascontiguousarray(
        f("bq").reshape(L, HP, 2 * D).transpose(0, 2, 1))   # [L,128,6]
    sh["bk"] = np.ascontiguousarray(
        f("bk").reshape(L, HP, 2 * D).transpose(0, 2, 1))
    sh["bv"] = np.ascontiguousarray(f("bv").reshape(L, 1, E)).astype(bf)
    sh["bo"] = np.ascontiguousarray(f("bo").reshape(L, EC, 128).transpose(0, 2, 1))
    sh["b1"] = np.ascontiguousarray(f("b1").reshape(L, FC, 128).transpose(0, 2, 1))
    sh["b2"] = np.ascontiguousarray(f("b2").reshape(L, EC, 128).transpose(0, 2, 1))
    for nm, key in (("g1", "ln1_g"), ("be1", "ln1_b"), ("g2", "ln2_g"), ("be2", "ln2_b")):
        sh[nm] = np.ascontiguousarray(
            f(key).reshape(L, EC, 128).transpose(0, 2, 1))
    sh["ident"] = np.eye(128, dtype=np.float32)
    sh["onesr"] = np.ones((128, 128), dtype=ml_dtypes.bfloat16)
    sh["onesh"] = np.ones((128, H), dtype=ml_dtypes.bfloat16)
    sh["epsb"] = np.full((128, 1), E * EPS, dtype=np.float32)
    return sh


def _core_idx(x, core):
    """Token-id tiles for one core: [10,128] int32."""
    ids = np.zeros((2 * 5, 128), dtype=np.int32)
    for b in range(BP):
        seq = np.asarray(x[core * BP + b]).astype(np.int64)
        for k in range(5):
            rows = min(128, T - k * 128)
            ids[b * 5 + k, :rows] = seq[k * 128:k * 128 + rows]
    return ids


_CACHE = {}


def kernel(**inputs) -> np.ndarray:
    from concourse.bass_utils import run_bass_kernel_spmd
    key = "nc"
    if key not in _CACHE:
        _CACHE[key] = build(nl=L, hw_loop=True)
    nc = _CACHE[key]
    sh = _marshal_shared(inputs, L)
    x = np.asarray(inputs["x"])
    in_maps = [dict(sh, idx=_core_idx(x, c)) for c in range(NCORE)]
    res = run_bass_kernel_spmd(nc, in_maps, core_ids=list(range(NCORE)))
    out = np.stack([r["out"] for r in res.results])        # [8, 1026, 768]
    return out.reshape(B, T, E).astype(np.float32)

